# revision 57
# baseline (speedup 1.0000x reference)
"""Trainium2 Bass kernel for CheemsMambaMixer (Mamba-1 selective SSM mixer).

Shapes: B=1, L=2048, H=1024, DI=2048, DS=16, DTR=64, K=4.
Sharding: tensor-parallel over the d_inner channel dim (256 channels/core on
8 cores).  The only cross-core communication is a [96, L] fp16 AllReduce of
the x_proj partial products (split in two L/2 chunks so it overlaps compute);
the out_proj row-parallel partials are summed on the host.

Everything device-side runs in fp16 storage with fp32 accumulation (PSUM,
scan state).  The kernel is organised as one software pipeline over L-halves:

  load chunk DMAs -> in_proj(x) -> conv+silu -> x_proj -> AllReduce(half)
    -> softplus(dt) -> per-(d,n)-tile scan (two chained half-scans)
    -> y reduction -> gate -> out_proj

so the recurrent-scan phase (DVE-bound) starts as early as possible and all
five engines stay busy.
"""
import sys

sys.path.insert(0, "/opt/trn_rl_repo")

import numpy as np

import concourse.bass as bass
import concourse.tile as tile
from concourse import mybir
from concourse.bass_utils import run_bass_kernel_spmd
from concourse.tile_rust import add_dep_helper
import bass_rust as _bass_rust

# ---------------------------------------------------------------- constants
N_CORES = 8
B, L, H = 1, 2048, 1024
DI, DS, DTR, K = 2048, 16, 64, 4
DIL = DI // N_CORES          # 256 channels per core
NDT = DIL // 128             # 2 d-tiles of 128 channels
LC = 512                     # time chunk
LH = 1024                    # time half
NCH = L // LC                # 4 chunks
NTILE = DIL * DS // 128      # 32 (d,n)-tiles per core, 8 d x 16 n each
TPG = NTILE // NDT           # 16 tiles per d-group

F16 = mybir.dt.float16
F32 = mybir.dt.float32

# Use the native Sigmoid activation for silu (needs a second ACT table set
# loaded at runtime) instead of synthesizing sigma via exp/ln in the single
# natural_log_exp table.
USE_SIGMOID = True

N_PROCS = 27


class _SplitDrainTileContext(tile.TileContext):
    """Tail drain split into single-wait drains: the CTRL_NO ISA struct holds
    one sync-wait, but a kernel using all 8 HWDGE queues plus a collective
    accumulates 9+ outstanding procs at the tail."""

    def _drain_and_barrier(self, tick_clock, wait_clock):
        full = tick_clock.global_clock
        ticks = [(i, full.peek_next(i) - 1) for i in range(N_PROCS)]
        ticks = [(i, v) for i, v in ticks if v > 0]
        for i, v in ticks:
            c = _bass_rust.VectorClock()
            c.require_at_least(i, v)
            drain_inst = self.nc.sync.drain(fusable=False)
            wait_clock.add_sem_waits(
                drain_inst.ins, _bass_rust.ScopedClock({None: c}))
        self.nc.all_engine_barrier()
        assert self.sems is not None
        popped = self.nc._tile_sem_poison_stack.pop()
        assert popped is self._sem_poison
        self.nc.clear_and_free_semaphores(list(self.sems.allocated().values()))
        self.nc.all_engine_barrier()


def _split_multi_waits(nc):
    """TPB ISA structs carry a single sync-wait slot; Tile sometimes attaches
    several.  Hoist all but the last wait of every instruction onto dedicated
    single-wait NoOps on the same engine, inserted just before it."""
    wid = 0
    for bb in nc.main_func.blocks:
        insts = list(bb.instructions)
        out = []
        changed = False
        for ins in insts:
            si = ins.sync_info
            if si is not None and si.on_wait and len(si.on_wait) > 1:
                waits = list(si.on_wait)
                for w in waits[:-1]:
                    nop = _bass_rust.InstNoOp(name=f"W-split-{wid}", ins=[],
                                              outs=[])
                    wid += 1
                    nop.engine = ins.engine
                    nop.sync_info = mybir.SyncInfo(on_wait=[w], on_update=[])
                    out.append(nop)
                ins.sync_info = mybir.SyncInfo(on_wait=[waits[-1]],
                                               on_update=list(si.on_update or []))
                changed = True
            out.append(ins)
        if changed:
            bb.instructions = out


# ---------------------------------------------------------------- builder
def _build(single_core=False, use_sigmoid=None):
    if use_sigmoid is None:
        use_sigmoid = USE_SIGMOID
    nc = bass.Bass("TRN2", target_bir_lowering=False, debug=False,
                   num_devices=N_CORES)
    Act = mybir.ActivationFunctionType
    Op = mybir.AluOpType

    def din(name, shape, dtype=F16):
        return nc.dram_tensor(name, shape, dtype, kind="ExternalInput").ap()

    hsT = din("hsT", [H, L])                       # hidden_states[0].T
    wxzT = din("wxzT", [H, 2 * DIL])               # in_proj rows (x|z).T slice
    owT = din("owT", [DIL, H])                     # out_proj.T slice
    xpwT = din("xpwT", [DIL, 96])                  # x_proj.T slice
    dtwT = din("dtwT", [DTR, DIL])                 # dt_proj.T slice
    selrep = din("selrep", [128, TPG, 128])        # SelRep[k, li, p]
    selmap = din("selmap", [128, TPG, 128])        # Selmap[p, li, m]
    selbc = din("selbc", [96, 2, 128])             # SelB / SelC
    acols = din("acols", [128, NTILE], F32)        # A[d,n] per (tile, partition)
    convw = din("convw", [128, NDT, K], F32)
    convb = din("convb", [128, NDT], F32)
    dtb = din("dtb", [128, NDT], F32)
    ddiag = din("ddiag", [128, NDT, 128])   # diag(D) per d-group
    out = nc.dram_tensor("out", [L, H], F16, kind="ExternalOutput").ap()

    hsT_r = hsT.rearrange("(k p) t -> p k t", p=128)
    wxzT_r = wxzT.rearrange("(k p) m -> p k m", p=128)

    with _SplitDrainTileContext(nc) as tc:
        import contextlib
        stack = contextlib.ExitStack()
        with stack:
            wpool = stack.enter_context(tc.tile_pool(name="wpool", bufs=1))
            state = stack.enter_context(tc.tile_pool(name="state", bufs=1))
            work = stack.enter_context(tc.tile_pool(name="work", bufs=3))
            psum = stack.enter_context(
                tc.tile_pool(name="psum", bufs=2, space="PSUM"))
            dram = stack.enter_context(
                tc.tile_pool(name="dram", bufs=1, space="DRAM"))

            # ---------------- load weights/constants, ordered by first use:
            # tiny conv/bias tensors first (the conv gates the whole x-chain
            # and must not queue behind the 1MB hsT chunks), then the x-path
            # weights, then later-phase tensors.
            convw_sb = wpool.tile([128, NDT, K], F32)
            convw_dma = nc.sync.dma_start(convw_sb, convw)
            convb_sb = wpool.tile([128, NDT], F32)
            convb_dma = nc.sync.dma_start(convb_sb, convb)
            xpwT_sb = wpool.tile([128, NDT, 96], F16)
            nc.sync.dma_start(xpwT_sb, xpwT.rearrange("(k p) j -> p k j", p=128))
            dtb_sb = wpool.tile([128, NDT], F32)
            nc.sync.dma_start(dtb_sb, dtb)
            wxzT_sb = wpool.tile([128, H // 128, 2 * DIL], F16)
            nc.sync.dma_start(wxzT_sb[:, :, 0:DIL], wxzT_r[:, :, 0:DIL])
            hsT_sb = [wpool.tile([128, H // 128, LC], F16, name=f"hsT{c}")
                      for c in range(NCH)]
            hs_dmas = [nc.sync.dma_start(hsT_sb[c],
                                         hsT_r[:, :, LC * c:LC * (c + 1)])
                       for c in range(NCH)]
            dtwT_sb = wpool.tile([DTR, NDT, 128], F16)
            nc.sync.dma_start(dtwT_sb, dtwT.rearrange("k (m p) -> k m p", p=128))
            selbc_sb = wpool.tile([96, 2, 128], F16)
            nc.sync.dma_start(selbc_sb, selbc)
            selrep_sb = wpool.tile([128, TPG, 128], F16)
            nc.sync.dma_start(selrep_sb, selrep)
            acols_sb = wpool.tile([128, NTILE], F32)
            nc.sync.dma_start(acols_sb, acols)
            nc.sync.dma_start(wxzT_sb[:, :, DIL:2 * DIL],
                              wxzT_r[:, :, DIL:2 * DIL])
            selmap_sb = wpool.tile([128, TPG, 128], F16)
            nc.sync.dma_start(selmap_sb, selmap)
            ddiag_sb = wpool.tile([128, NDT, 128], F16)
            nc.sync.dma_start(ddiag_sb, ddiag)
            owT_sb = wpool.tile([128, NDT, H], F16)
            nc.sync.dma_start(owT_sb, owT.rearrange("(k p) h -> p k h", p=128))

            # Wait-slot fencing: TensorScalarPtr-class DVE ops (tensor_scalar,
            # scalar_tensor_tensor, tensor_tensor_scan) have very few sync-wait
            # slots in their ISA structs.  A tiny TensorTensor op (2 wait
            # slots) placed just before makes the in-order DVE observe the
            # producers' semaphores so the fragile op needs no new waits.
            fence_scratch = wpool.tile([128, 4], F32)

            def dve_observe(*insts):
                insts = [i for i in insts if i is not None]
                for j in range(0, len(insts), 1):
                    f = nc.vector.tensor_mul(
                        fence_scratch[:, 0:1], fence_scratch[:, 0:1],
                        fence_scratch[:, 0:1])
                    for d in insts[j:j + 1]:
                        add_dep_helper(f.ins, d.ins, sync=True,
                                       reason="dve wait fence")

            # ACT spline tables: no single set has exp+ln+sigmoid, but bacc's
            # insert_act_table_loads pass supports multiple sets (1.3us per
            # load).  All Sigmoid ops are emitted before the first Exp/Ln op
            # (Copy lives in every set), so exactly two table loads happen:
            # sigmoid_and_others first, then natural_log_exp_and_others for
            # softplus = ln(1 + e^x) and the scan's dA = exp(A*dt).

            # persistent state tensors
            xc = [state.tile([128, L], F16, name=f"xc{i}") for i in range(NDT)]
            zsb = [state.tile([128, L], F16, name=f"zsb{i}") for i in range(NDT)]
            dt = [state.tile([128, L], F16, name=f"dt{i}") for i in range(NDT)]
            dtx = [state.tile([128, L], F16, name=f"dtx{i}") for i in range(NDT)]
            ssm16 = state.tile([96, L], F16, name="ssm16")
            ssmr16 = state.tile([96, L], F16, name="ssmr16")
            brep = state.tile([128, L], F16, name="brep")
            crep = state.tile([128, L], F16, name="crep")
            ysb = [state.tile([128, L], F16, name=f"ysb{g}") for g in range(NDT)]
            yg = [state.tile([128, L], F16, name=f"yg{g}") for g in range(NDT)]
            xpad = [state.tile([128, K - 1 + L], F16, name=f"xpad{i}")
                    for i in range(NDT)]
            sig = [state.tile([128, L], F16, name=f"sig{i}") for i in range(NDT)]

            for i in range(NDT):
                nc.vector.memset(xpad[i][:, 0:K - 1], 0.0)

            # ---------------- phase 1: in_proj(x) + conv + silu + x_proj,
            # pipelined per L-half; z-branch interleaved into PE gaps.
            x_evacs = {}

            def x_mm(tch):
                for dm in range(NDT):
                    # pre-scan in_proj rotates through the 4 "yac" banks
                    # (unused until the scan) so PE never stalls on PSUM WAR
                    ps = psum.tile([128, LC], F32, tag="yac", bufs=4)
                    for k in range(H // 128):
                        nc.tensor.matmul(
                            ps,
                            lhsT=wxzT_sb[:, k, 128 * dm:128 * (dm + 1)],
                            rhs=hsT_sb[tch][:, k, :],
                            start=(k == 0), stop=(k == H // 128 - 1))
                    # evacuate on ACT (idle in the early ramp; DVE runs the
                    # conv and is the scarcer engine)
                    x_evacs[(tch, dm)] = nc.scalar.copy(
                        xpad[dm][:, K - 1 + LC * tch:K - 1 + LC * (tch + 1)],
                        ps)

            def z_mm1(tch, dm):
                # fills the AllReduce-latency PE gaps before the scan tiles
                ps = psum.tile([128, LC], F32, tag="yac", bufs=4)
                for k in range(H // 128):
                    nc.tensor.matmul(
                        ps,
                        lhsT=wxzT_sb[:, k, 128 * (NDT + dm):
                                     128 * (NDT + dm + 1)],
                        rhs=hsT_sb[tch][:, k, :],
                        start=(k == 0), stop=(k == H // 128 - 1))
                nc.vector.tensor_copy(zsb[dm][:, LC * tch:LC * (tch + 1)], ps)

            def conv_silu_x(dm, tch):
                """causal depthwise conv (K=4) + bias + silu for one L/4
                chunk -- quarter granularity keeps the chunk-0 x_proj (and so
                AllReduce 0) as early as possible."""
                csl = slice(LC * tch, LC * (tch + 1))
                acc = work.tile([128, LC], F16, tag="convacc", bufs=2,
                                name="acc")
                # the fragile TensorScalarPtr conv ops read the ACT-written
                # xpad chunk: absorb that wait into a fence first
                dve_observe(x_evacs[(tch, dm)])
                nc.vector.tensor_scalar(
                    acc, xpad[dm][:, LC * tch:LC * tch + LC],
                    convw_sb[:, dm, 0:1], convb_sb[:, dm:dm + 1],
                    op0=Op.mult, op1=Op.add)
                for k in range(1, K):
                    nc.vector.scalar_tensor_tensor(
                        acc, xpad[dm][:, LC * tch + k:LC * tch + k + LC],
                        convw_sb[:, dm, k:k + 1], acc, op0=Op.mult, op1=Op.add)
                sigmoid_into(sig[dm][:, csl], acc)
                nc.vector.tensor_mul(xc[dm][:, csl], acc, sig[dm][:, csl])

            def sigmoid_into(dst, v, force_expln=False):
                """dst = sigma(v): native table op, or synthesized as
                sigma(v) = exp(-ln(1+e^-v)) inside the exp/ln table set."""
                if use_sigmoid and not force_expln:
                    nc.scalar.activation(dst, v, Act.Sigmoid)
                    return
                w = v.shape[-1]
                t1 = work.tile([128, w], F16, tag="zs_t1", bufs=1, name="t1")
                nc.scalar.activation(t1, v, Act.Exp, scale=-1.0)
                t2 = work.tile([128, w], F16, tag="zs_t2", bufs=1, name="t2")
                nc.scalar.activation(t2, t1, Act.Ln, bias=1.0)
                nc.scalar.activation(dst, t2, Act.Exp, scale=-1.0)

            def z_silu(dm, h, force_expln=False):
                hsl = slice(LH * h, LH * (h + 1))
                sigmoid_into(sig[dm][:, hsl], zsb[dm][:, hsl], force_expln)
                nc.vector.tensor_mul(zsb[dm][:, hsl], zsb[dm][:, hsl],
                                     sig[dm][:, hsl])

            def z_silu_expln(dm, h):
                # inside the scan's exp/ln table region: never use Sigmoid
                z_silu(dm, h, force_expln=True)

            def xproj(tch):
                ps = psum.tile([128, LC], F32, tag="yac", bufs=4,
                               name="ssm_ps")
                for ki in range(NDT):
                    nc.tensor.matmul(
                        ps[0:96, :], lhsT=xpwT_sb[:, ki, :],
                        rhs=xc[ki][:, LC * tch:LC * (tch + 1)],
                        start=(ki == 0), stop=(ki == NDT - 1))
                nc.vector.tensor_copy(ssm16[:, LC * tch:LC * (tch + 1)],
                                      ps[0:96, :])

            ar_in = [dram.tile([96, LH], F16, name=f"ar_in{h}")
                     for h in range(2)]
            ar_out = [dram.tile([96, LH], F16, name=f"ar_out{h}")
                      for h in range(2)]

            def allreduce(h):
                hsl = slice(LH * h, LH * (h + 1))
                nc.sync.dma_start(ar_in[h], ssm16[:, hsl])
                if single_core:
                    nc.sync.dma_start(ar_out[h], ar_in[h])
                else:
                    nc.gpsimd.collective_compute(
                        "AllReduce", Op.add,
                        replica_groups=[list(range(N_CORES))],
                        ins=[ar_in[h].opt()], outs=[ar_out[h].opt()])
                return nc.sync.dma_start(ssmr16[:, hsl], ar_out[h])

            # emission order builds the per-engine streams: the chunk-0/1
            # chain (in_proj x -> conv -> silu -> x_proj -> AllReduce 0) is
            # emitted first so AR0 flies at ~20us; the z-branch matmuls are
            # deferred into the scan phase where PE has ~45% slack; z-silu
            # runs as one Sigmoid block mid-scan (two extra table loads).
            x_mm(0)
            dve_observe(convw_dma, convb_dma)
            conv_silu_x(0, 0)
            conv_silu_x(1, 0)
            x_mm(1)
            conv_silu_x(0, 1)
            conv_silu_x(1, 1)
            xproj(0)
            xproj(1)
            ar0 = allreduce(0)
            x_mm(2)
            conv_silu_x(0, 2)
            conv_silu_x(1, 2)
            x_mm(3)
            conv_silu_x(0, 3)
            conv_silu_x(1, 3)
            xproj(2)
            xproj(3)
            ar1 = allreduce(1)

            # ---------------- phase 2: dt = softplus(dt_proj @ dtr + b); dtx;
            # B_rep / C_rep -- all gated per AllReduce half.
            def dt_softplus(mi, tch):
                ps = psum.tile([128, LC], F32, tag="yac", bufs=4,
                               name="dt_ps")
                nc.tensor.matmul(
                    ps, lhsT=dtwT_sb[:, mi, :],
                    rhs=ssmr16[0:DTR, LC * tch:LC * (tch + 1)],
                    start=True, stop=True)
                # softplus(x+b) = ln(1 + e^(x+b)) via the exp/ln table set
                spe = work.tile([128, LC], F32, tag="spe", bufs=2, name="spe")
                nc.scalar.activation(spe, ps, Act.Exp,
                                     bias=dtb_sb[:, mi:mi + 1])
                nc.scalar.activation(
                    dt[mi][:, LC * tch:LC * (tch + 1)], spe, Act.Ln, bias=1.0)

            def bc_rep(tch):
                for j, dest in ((0, brep), (1, crep)):
                    ps = psum.tile([128, LC], F32, tag="yac", bufs=4,
                                   name="bc_ps")
                    nc.tensor.matmul(ps, lhsT=selbc_sb[:, j, :],
                                     rhs=ssmr16[:, LC * tch:LC * (tch + 1)],
                                     start=True, stop=True)
                    nc.vector.tensor_copy(dest[:, LC * tch:LC * (tch + 1)],
                                          ps)

            # dt gates the scan's dA build directly, so softplus goes ahead
            # of the B/C broadcasts in the ACT queue; z matmuls fill the
            # AllReduce-latency PE gaps.
            for h in range(2):
                for tch in (2 * h, 2 * h + 1):
                    for mi in range(NDT):
                        dt_softplus(mi, tch)
                for mi in range(NDT):
                    hsl = slice(LH * h, LH * (h + 1))
                    dve_observe(ar0 if h == 0 else ar1)
                    nc.vector.tensor_mul(dtx[mi][:, hsl], dt[mi][:, hsl],
                                         xc[mi][:, hsl])
                for tch in (2 * h, 2 * h + 1):
                    bc_rep(tch)
                for tch in (2 * h, 2 * h + 1):
                    for dm in range(NDT):
                        z_mm1(tch, dm)
            # z-silu as one contiguous Sigmoid block (bracketed by two
            # ACT-table loads) before the scan's exp stream begins
            for dm in range(NDT):
                z_silu(dm, 0)
                z_silu(dm, 1)

            # ---------------- phase 3: the scan
            # Per (d,n)-tile: build dA/dBx per L-half, run two chained
            # tensor_tensor_scans, multiply by C, reduce n via selection
            # matmuls accumulating dense y per chunk bank.
            for g in range(NDT):
                yac = [psum.tile([128, LC], F32, tag="yac", bufs=4,
                                 name=f"yac{c}") for c in range(NCH)]
                for li in range(TPG):
                    i = TPG * g + li
                    # the first two tiles run as two chained half-scans so the
                    # scan starts before AllReduce half 1 has landed; later
                    # tiles use one full-L scan (cheaper per element).
                    split = (g == 0 and li < 2)
                    dA = work.tile([128, L], F16, tag="dA", bufs=2)
                    dBx = work.tile([128, L], F16, tag="dBx", bufs=2)
                    # deep hv/hc rings decouple the DVE scan rate from the
                    # slower Pool C-multiply consuming hv
                    hv = work.tile([128, L], F16, tag="hv", bufs=4)
                    hc = work.tile([128, L], F16, tag="hc", bufs=4)
                    for h in range(2):
                        hsl = slice(LH * h, LH * (h + 1))
                        # drep and dxp alternate through one [128,1024] PSUM
                        # ring (4 banks total); one merged exp per half.
                        drep = psum.tile([128, LH], F32, tag="dxrep", bufs=2,
                                         name="drep")
                        for cc in range(2):
                            nc.tensor.matmul(
                                drep[:, LC * cc:LC * (cc + 1)],
                                lhsT=selrep_sb[:, li, :],
                                rhs=dt[g][:, LH * h + LC * cc:
                                          LH * h + LC * (cc + 1)],
                                start=True, stop=True)
                        nc.scalar.activation(dA[:, hsl], drep, Act.Exp,
                                             scale=acols_sb[:, i:i + 1])
                        # dBx = broadcast(dtx) * brep: ACT evacuates the fp32
                        # PSUM broadcast to fp16 (a fp32 operand would force
                        # the multiply into DVE 1x mode), DVE multiplies in
                        # 2x mode.
                        dxp = psum.tile([128, LH], F32, tag="dxrep", bufs=2,
                                        name="dxp")
                        for cc in range(2):
                            nc.tensor.matmul(
                                dxp[:, LC * cc:LC * (cc + 1)],
                                lhsT=selrep_sb[:, li, :],
                                rhs=dtx[g][:, LH * h + LC * cc:
                                           LH * h + LC * (cc + 1)],
                                start=True, stop=True)
                        if g == 0 and li < 2:
                            # ramp region: ACT is the backlogged engine, DVE
                            # mostly idle -- multiply straight from fp32 PSUM
                            # (DVE 1x mode) instead of evacuating on ACT
                            nc.vector.tensor_mul(dBx[:, hsl], dxp,
                                                 brep[:, hsl])
                        else:
                            dxp16 = work.tile([128, LH], F16, tag="dxp16",
                                              bufs=2, name="dxp16")
                            nc.scalar.copy(dxp16, dxp)
                            nc.vector.tensor_mul(dBx[:, hsl], dxp16,
                                                 brep[:, hsl])
                        if split:
                            nc.vector.tensor_tensor_scan(
                                hv[:, hsl], dA[:, hsl], dBx[:, hsl],
                                0.0 if h == 0 else hv[:, LH - 1:LH],
                                op0=Op.mult, op1=Op.add)
                            nc.vector.tensor_mul(hc[:, hsl], hv[:, hsl],
                                                 crep[:, hsl])
                            for c in (2 * h, 2 * h + 1):
                                tsl = slice(LC * c, LC * (c + 1))
                                nc.tensor.matmul(yac[c],
                                                 lhsT=selmap_sb[:, li, :],
                                                 rhs=hc[:, tsl],
                                                 start=(li == 0), stop=False)
                    if not split:
                        nc.vector.tensor_tensor_scan(
                            hv, dA, dBx, 0.0, op0=Op.mult, op1=Op.add)
                        nc.vector.tensor_mul(hc, hv, crep)
                        for c in range(NCH):
                            tsl = slice(LC * c, LC * (c + 1))
                            nc.tensor.matmul(yac[c], lhsT=selmap_sb[:, li, :],
                                             rhs=hc[:, tsl],
                                             start=(li == 0), stop=False)
                for c in range(NCH):
                    tsl = slice(LC * c, LC * (c + 1))
                    # y += D * xc folded in as a diagonal matmul
                    nc.tensor.matmul(yac[c], lhsT=ddiag_sb[:, g, :],
                                     rhs=xc[g][:, tsl], start=False, stop=True)
                    nc.scalar.copy(ysb[g][:, tsl], yac[c])
                    # gating per chunk so out_proj can start early
                    nc.vector.tensor_mul(yg[g][:, tsl], ysb[g][:, tsl],
                                         zsb[g][:, tsl])

            # ---------------- phase 4: out_proj
            # out PSUM rotates through the freed yac banks (4-deep) so the
            # matmul->evac->DMA chain pipelines; evacs alternate ACT/DVE; one
            # batched DMA per 128-row block.
            for tb in range(L // 128):
                osb = work.tile([128, H], F16, tag="osb", bufs=4)
                for hch in range(H // LC):
                    ops = psum.tile([128, LC], F32, tag="yac", bufs=4,
                                    name="out_ps")
                    for g in range(NDT):
                        nc.tensor.matmul(
                            ops, lhsT=yg[g][:, 128 * tb:128 * (tb + 1)],
                            rhs=owT_sb[:, g, LC * hch:LC * (hch + 1)],
                            start=(g == 0), stop=(g == NDT - 1))
                    dst = osb[:, LC * hch:LC * (hch + 1)]
                    if hch % 2 == 0:
                        nc.scalar.copy(dst, ops)
                    else:
                        nc.vector.tensor_copy(dst, ops)
                nc.sync.dma_start(out[128 * tb:128 * (tb + 1), :], osb)
    _split_multi_waits(nc)
    return nc


_NC_CACHE = None


def _get_nc():
    global _NC_CACHE
    if _NC_CACHE is None:
        _NC_CACHE = _build()
    return _NC_CACHE


# ---------------------------------------------------------------- host side
def _make_in_maps(hidden_states, in_proj_w, conv_w, conv_b, x_proj_w,
                  dt_proj_w, dt_proj_b, A_log, D, out_proj_w):
    hsT16 = np.ascontiguousarray(hidden_states[0].T, dtype=np.float16)

    # selection matrices (shared by all cores)
    p = np.arange(128)
    li = np.arange(TPG)
    k = np.arange(128)
    # SelRep[k, li, p] = 1 iff k == 8*li + p//16
    selrep = (k[:, None, None] == 8 * li[None, :, None] +
              (p // 16)[None, None, :]).astype(np.float16)
    # Selmap[p, li, m] = 1 iff m == 8*li + p//16
    selmap = (k[None, None, :] == 8 * li[None, :, None] +
              (p // 16)[:, None, None]).astype(np.float16)
    k96 = np.arange(96)
    selb = (k96[:, None] == 64 + (p % 16)[None, :])
    selc = (k96[:, None] == 80 + (p % 16)[None, :])
    selbc = np.stack([selb, selc], axis=1).astype(np.float16)

    A = -np.exp(np.asarray(A_log, np.float64))     # [DI, DS]

    in_maps = []
    for c in range(N_CORES):
        s = slice(DIL * c, DIL * (c + 1))
        wxz = np.concatenate(
            [in_proj_w[s], in_proj_w[DI + DIL * c:DI + DIL * (c + 1)]], axis=0)
        Ac = A[s]                                   # [256, 16]
        ti = np.arange(NTILE)
        acols = Ac[8 * ti[None, :] + (p // 16)[:, None], (p % 16)[:, None]]
        in_maps.append({
            "hsT": hsT16,
            "wxzT": np.ascontiguousarray(wxz.T, dtype=np.float16),
            "owT": np.ascontiguousarray(out_proj_w[:, s].T, dtype=np.float16),
            "xpwT": np.ascontiguousarray(x_proj_w[:, s].T, dtype=np.float16),
            "dtwT": np.ascontiguousarray(dt_proj_w[s].T, dtype=np.float16),
            "selrep": selrep, "selmap": selmap, "selbc": selbc,
            "acols": np.ascontiguousarray(acols, np.float32),
            "convw": np.ascontiguousarray(
                conv_w[s, 0, :].reshape(NDT, 128, K).transpose(1, 0, 2),
                np.float32),
            "convb": np.ascontiguousarray(
                conv_b[s].reshape(NDT, 128).T, np.float32),
            "dtb": np.ascontiguousarray(
                dt_proj_b[s].reshape(NDT, 128).T, np.float32),
            "ddiag": np.ascontiguousarray(
                np.einsum("gp,pm->pgm", D[s].reshape(NDT, 128),
                          np.eye(128)), np.float16),
        })
    return in_maps


def kernel(hidden_states, in_proj_w, conv_w, conv_b, x_proj_w,
           dt_proj_w, dt_proj_b, A_log, D, out_proj_w):
    args = [np.asarray(a, np.float32) for a in
            (hidden_states, in_proj_w, conv_w, conv_b, x_proj_w,
             dt_proj_w, dt_proj_b, A_log, D, out_proj_w)]
    in_maps = _make_in_maps(*args)
    nc = _get_nc()
    res = run_bass_kernel_spmd(nc, in_maps, core_ids=list(range(N_CORES)))
    out = np.zeros((L, H), np.float64)
    for r in res.results:
        out += r["out"].astype(np.float64)
    return out.astype(np.float32).reshape(B, L, H)
